# revision 6
# baseline (speedup 1.0000x reference)
"""Multi-head attention (B=4, S=2048, D=1024, H=16) on 8 TRN2 NeuronCores.

Sharding: data-parallel over batch (4) x tensor-parallel over head halves (2).
Core c handles batch b = c//2 and heads [8g, 8g+8) where g = c%2.
Each core computes a partial [S, D] output-projection contribution; the host
sums the two head-group partials per batch.

Layout (all matmul operands bf16, fp32 PSUM accumulation):
  - activations arrive host-transposed (feature dim on partitions),
  - scores are built transposed [k, q]; one PSUM tile [128, 1024] holds the
    scores of BOTH heads of a pair ([k, head0 q | head1 q]) so the two
    DK=64 score matmuls (PE row halves 0-63 / 64-127, tile_position (0,0)
    and (64,0)) become ready together, schedule back-to-back, and run
    CONCURRENTLY on the PE array (row tiling),
  - exp() covers both heads in one [128, 1024] activation instruction,
  - P @ [V | 1] yields the softmax denominator as row 64 of U^T for free,
  - normalized U^T tiles are exactly the stationary layout w_o needs.

Scheduling: the attention stream (score MMs -> exp -> PV MMs) is emitted at
normal priority; every projection/output-projection "piece" is emitted at
LOW priority (tc.high_priority with a negative offset) so the Tile
scheduler treats pieces as pure PE gap-fillers — the scalar engine's exp
stream (~266us busy) never waits behind piece work.

Phase A processes q-chunks {0,1} interleaved c-major so the K/V projection
of chunks 1-3 fits under 2x of exp cover; phase B runs q-chunks 2,3
qc-major with the w_o pieces of earlier chunks as fillers. The final
chunk's w_o runs at quarter width (N=256) to shorten the serial tail.

PSUM budget (8 banks): 2 x sc [128,1024] (4 banks) + 2 x Up [65,512]
(2 banks) + 2 x piece accumulators [128,512] (2 banks).
"""

import numpy as np

B, S, D, H = 4, 2048, 1024, 16
DK = D // H          # 64
G = 2                # head groups (tensor-parallel degree per batch)
HL = H // G          # 8 local heads per core
DV = HL * DK         # 512 local value dim
N_CORES = 8

LOWP = -1_000_000    # priority offset for gap-filler pieces

_cached = {}


def _build():
    import concourse.bass as bass
    import concourse.tile as tile
    from concourse import bacc, mybir

    f32 = mybir.dt.float32
    bf16 = mybir.dt.bfloat16
    EXP = mybir.ActivationFunctionType.Exp

    nc = bacc.Bacc("TRN2", target_bir_lowering=False, debug=False,
                   num_devices=N_CORES)

    xqT = nc.dram_tensor("xqT", [D, S], bf16, kind="ExternalInput").ap()
    xkT = nc.dram_tensor("xkT", [D, S], bf16, kind="ExternalInput").ap()
    xvT = nc.dram_tensor("xvT", [D, S], bf16, kind="ExternalInput").ap()
    wqT = nc.dram_tensor("wqT", [D, DV], bf16, kind="ExternalInput").ap()
    wkT = nc.dram_tensor("wkT", [D, DV], bf16, kind="ExternalInput").ap()
    wvT = nc.dram_tensor("wvT", [D, DV], bf16, kind="ExternalInput").ap()
    woT = nc.dram_tensor("woT", [DV, D], bf16, kind="ExternalInput").ap()
    out = nc.dram_tensor("out", [S, D], f32, kind="ExternalOutput").ap()

    ND = D // 128     # 8 d-tiles
    NS = S // 128     # 16 k-tiles
    NQC = S // 512    # 4 q-chunks
    NT = DV // 128    # 4 dk/dv-tiles
    NHP = HL // 2     # 4 head pairs

    with tile.TileContext(nc) as tc:
        with (
            tc.tile_pool(name="persist", bufs=1) as persist,
            tc.tile_pool(name="stage", bufs=32) as stage,
            tc.tile_pool(name="wpool", bufs=8) as wpool,
            tc.tile_pool(name="spool", bufs=2, space=bass.MemorySpace.PSUM) as spool,
            tc.tile_pool(name="upool", bufs=2, space=bass.MemorySpace.PSUM) as upool,
            tc.tile_pool(name="gpool", bufs=2, space=bass.MemorySpace.PSUM) as gpool,
            tc.tile_pool(name="ppool", bufs=4) as ppool,
            tc.tile_pool(name="rpool", bufs=3) as rpool,
            tc.tile_pool(name="obuf", bufs=3) as obuf,
        ):
            QT = {}    # [t][qc] -> [128, 512] tiles of Q^T (dk rows, q cols)
            KT = {}    # [t][c]  -> [128, 512]
            Vaug = {}  # [kt] -> [128, 8, 65]: per-head V columns + ones col
            outT = {}  # [qc][t] -> [128, 512] normalized attention out^T
            wks, wvs, wqs, wos = [], [], [], []
            st_ = {}   # per (qc, hp) attention state
            xq_stage = {}

            def uacc():
                return upool.tile([65, 512], f32, tag="u", name="uacc")

            def gacc():
                return gpool.tile([128, 512], f32, tag="g", name="gacc")

            def emit_w_loads(lst, name, src):
                for d in range(ND):
                    wt = wpool.tile([128, DV], bf16, tag="w", name=name,
                                    bufs=24)
                    nc.sync.dma_start(wt[:], src[128 * d:128 * (d + 1), :])
                    lst.append(wt)

            def emit_wo_loads():
                for t in range(NT):
                    wo = wpool.tile([128, D], bf16, tag=f"wo{t}", name="wo",
                                    bufs=1)
                    nc.sync.dma_start(wo[:], woT[128 * t:128 * (t + 1), :])
                    wos.append(wo)

            def emit_x_dmas(src, c, name):
                xs = []
                for d in range(ND):
                    xt = stage.tile([128, 512], bf16, tag="act", name=name)
                    nc.sync.dma_start(
                        xt[:], src[128 * d:128 * (d + 1), 512 * c:512 * (c + 1)])
                    xs.append(xt)
                return xs

            def piece_kproj(xks, c, t):
                def go():
                    acc = gacc()
                    for d in range(ND):
                        nc.tensor.matmul(
                            acc[:], wks[d][:, 128 * t:128 * (t + 1)], xks[d][:],
                            start=(d == 0), stop=(d == ND - 1))
                    dt_ = persist.tile([128, 512], bf16, tag=f"kT{t}_{c}",
                                       name="kT")
                    nc.vector.tensor_copy(dt_[:], acc[:])
                    KT.setdefault(t, {})[c] = dt_
                return go

            def piece_vproj(xvs, c, ktl):
                def go():
                    kt = 4 * c + ktl
                    acc = gacc()
                    for d in range(ND):
                        nc.tensor.matmul(
                            acc[:], xvs[d][:, 128 * ktl:128 * (ktl + 1)],
                            wvs[d][:],
                            start=(d == 0), stop=(d == ND - 1))
                    va = persist.tile([128, HL, DK + 1], bf16, tag=f"vaug{kt}",
                                      name="vaug")
                    nc.vector.tensor_copy(
                        va[:, :, 0:DK],
                        acc[:].rearrange("p (h k) -> p h k", h=HL))
                    nc.vector.tensor_copy(
                        va[:, :, DK], nc.const_aps.tensor(1.0, (128, HL), bf16))
                    Vaug[kt] = va
                return go

            def emit_xq_dmas(qc):
                xs = []
                for d in range(ND):
                    xt = stage.tile([128, 512], bf16, tag="act", name="xq")
                    nc.sync.dma_start(
                        xt[:], xqT[128 * d:128 * (d + 1), 512 * qc:512 * (qc + 1)])
                    xs.append(xt)
                xq_stage[qc] = xs

            def piece_qproj(qc, t):
                def go():
                    xs = xq_stage[qc]
                    acc = gacc()
                    for d in range(ND):
                        nc.tensor.matmul(
                            acc[:], wqs[d][:, 128 * t:128 * (t + 1)], xs[d][:],
                            start=(d == 0), stop=(d == ND - 1))
                    dt_ = persist.tile([128, 512], bf16, tag=f"qT{t}_{qc}",
                                       name="qT")
                    nc.vector.tensor_copy(dt_[:], acc[:])
                    QT.setdefault(t, {})[qc] = dt_
                return go

            def piece_wo(qc, st, ncols, ncol_w):
                # final[s, n] = sum_dv outT[dv, s] * woT[dv, n]
                def go():
                    acc = gacc()
                    for t in range(NT):
                        nc.tensor.matmul(
                            acc[:, 0:ncol_w],
                            outT[qc][t][:, 128 * st:128 * (st + 1)],
                            wos[t][:, ncol_w * ncols:ncol_w * (ncols + 1)],
                            start=(t == 0), stop=(t == NT - 1))
                    ob = obuf.tile([128, 512], f32, tag="ob", name="ob")
                    nc.vector.tensor_copy(ob[:, 0:ncol_w], acc[:, 0:ncol_w])
                    nc.sync.dma_start(
                        out[512 * qc + 128 * st:512 * qc + 128 * (st + 1),
                            ncol_w * ncols:ncol_w * (ncols + 1)],
                        ob[:, 0:ncol_w])
                return go

            def lowp(fn):
                def go():
                    with tc.high_priority(offset=LOWP):
                        fn()
                return go

            def emit_attn_turn(qc, hp, c):
                # head pair (2hp, 2hp+1) = partition halves of tile hp. One
                # sc tile [128 kpos, head0 q | head1 q] per k-tile: the two
                # DK=64 score matmuls share the tile (ready together ->
                # adjacent in the PE queue -> concurrent row tiles).
                t = hp
                s = st_.setdefault((qc, hp), {})
                if c == 0:
                    s["Usb"] = [rpool.tile([65, 512], f32, tag=f"usb{hp}_{i}",
                                           name="usb", bufs=2)
                                for i in range(2)]
                Up = [None, None]
                for ktl in range(4):
                    kt = 4 * c + ktl
                    sc = spool.tile([128, 1024], f32, tag="sc", name="sc")
                    for i in range(2):
                        po = 64 * i
                        nc.tensor.matmul(
                            sc[:, 512 * i:512 * (i + 1)],
                            KT[t][c][po:po + 64,
                                     128 * ktl:128 * (ktl + 1)],
                            QT[t][qc][po:po + 64, :],
                            start=True, stop=True)
                    P = ppool.tile([128, 1024], bf16, tag="p", name="p")
                    nc.scalar.activation(P[:], sc[:], EXP, scale=0.125)
                    if ktl == 0:
                        Up[0] = uacc()
                        Up[1] = uacc()
                    for i in range(2):
                        nc.tensor.matmul(
                            Up[i][:],
                            Vaug[kt][:, 2 * hp + i, :],
                            P[:, 512 * i:512 * (i + 1)],
                            start=(ktl == 0), stop=(ktl == 3))
                for i in range(2):
                    if c == 0:
                        nc.vector.tensor_copy(s["Usb"][i][:], Up[i][:])
                    else:
                        nc.vector.tensor_add(s["Usb"][i][:],
                                             s["Usb"][i][:], Up[i][:])

            def emit_normalize(qc, hp):
                # rows 0..63 of U divided by row 64 (the ones-column sum),
                # written into out^T. Engine ops keep operands on one
                # partition range; cross-partition moves via SBUF-SBUF DMA.
                t = hp
                Usb = st_[(qc, hp)]["Usb"]
                ot = persist.tile([128, 512], bf16, tag=f"oT{t}_{qc % 2}",
                                  name="oT")
                outT.setdefault(qc, {})[t] = ot
                for i in range(2):
                    rrow = rpool.tile([1, 512], f32, tag="rrow", name="rrow")
                    nc.sync.dma_start(rrow[:], Usb[i][64:65, :])
                    rrec = rpool.tile([1, 512], f32, tag="rrec", name="rrec")
                    nc.vector.reciprocal_approx_fast(rrec[:], rrow[:])
                    rb = rpool.tile([64, 512], f32, tag="rb", name="rb")
                    nc.gpsimd.partition_broadcast(rb[:], rrec[:])
                    if i == 0:
                        nc.vector.tensor_mul(ot[0:64, :], Usb[i][0:64, :],
                                             rb[:])
                    else:
                        stg = rpool.tile([64, 512], bf16, tag="stg",
                                         name="stg")
                        nc.vector.tensor_mul(stg[:], Usb[i][0:64, :], rb[:])
                        nc.sync.dma_start(ot[64:128, :], stg[:])

            # ---- warm-up: load the exp table + flip the PE HAM to full
            # clock during the initial DMA wait, using a zeroed SBUF tile.
            # The exp reads the warm matmul's PSUM and the result lands in
            # `out` (overwritten later) so nothing here is dead code.
            wtile = stage.tile([128, 512], bf16, tag="warm", name="warm",
                               bufs=1)
            nc.vector.memset(wtile[:], 0.0)
            wacc = gacc()
            for r in range(16):
                nc.tensor.matmul(wacc[:], wtile[:, 0:128], wtile[:],
                                 start=(r == 0), stop=(r == 15))
            wexp = stage.tile([128, 512], f32, tag="warm2", name="warm2",
                              bufs=1)
            nc.scalar.activation(wexp[:], wacc[:], EXP, scale=0.125)
            nc.sync.dma_start(out[0:128, 0:512], wexp[:])

            # ---- DMA order: V path first (first PE work), then K, Q.
            emit_w_loads(wvs, "wv", wvT)
            xvs0 = emit_x_dmas(xvT, 0, "xv")
            emit_w_loads(wks, "wk", wkT)
            xks0 = emit_x_dmas(xkT, 0, "xk")
            emit_w_loads(wqs, "wq", wqT)
            emit_xq_dmas(0)
            emit_xq_dmas(1)

            xstage = {0: (xks0, xvs0)}

            # Pieces keyed for just-in-time emission before the turn that
            # first reads their tile (a Python-level ordering requirement;
            # execution order is still dependency + priority driven).
            emitted = set()

            def emit_piece(key):
                if key in emitted:
                    return
                emitted.add(key)
                kind = key[0]
                if kind == "v":
                    _, c, ktl = key
                    lowp(piece_vproj(xstage[c][1], c, ktl))()
                elif kind == "k":
                    _, c, t = key
                    lowp(piece_kproj(xstage[c][0], c, t))()
                elif kind == "q":
                    _, qc, t = key
                    lowp(piece_qproj(qc, t))()

            def turn_needs(qc, hp, c):
                return ([("v", c, ktl) for ktl in range(4)]
                        + [("k", c, hp), ("q", qc, hp)])

            # ---- prologue: minimal deps of the first attention turn at
            # normal priority; the other three Vaug tiles follow low-prio.
            piece_vproj(xvs0, 0, 0)()
            piece_kproj(xks0, 0, 0)()
            piece_qproj(0, 0)()
            emitted |= {("v", 0, 0), ("k", 0, 0), ("q", 0, 0)}

            # ---- phase A: q-chunks {0, 1} interleaved, c-major, so the
            # K/V projection of chunks 1-3 sits under 2x of exp cover.
            extras = [("q", 2, t) for t in range(NT)]
            for c in range(4):
                if c < 3:
                    xstage[c + 1] = (emit_x_dmas(xkT, c + 1, "xk"),
                                     emit_x_dmas(xvT, c + 1, "xv"))
                if c == 1:
                    emit_wo_loads()
                    emit_xq_dmas(2)
                    emit_xq_dmas(3)
                for hp in range(NHP):
                    for qc in (0, 1):
                        for key in turn_needs(qc, hp, c):
                            emit_piece(key)
                        emit_attn_turn(qc, hp, c)
                        if c == 3:
                            emit_normalize(qc, hp)
                        if c >= 2 and extras:
                            emit_piece(extras.pop(0))

            # ---- phase B: q-chunks 2 then 3, qc-major; w_o of earlier
            # chunks as fillers; final chunk's w_o at quarter width to
            # shorten the serial tail.
            fillers = {2: [(0, st2, ncol) for st2 in range(4)
                           for ncol in range(2)]
                       + [(1, st2, ncol) for st2 in range(4)
                          for ncol in range(2)],
                       3: [(2, st2, ncol) for st2 in range(4)
                           for ncol in range(2)]}
            extras = [("q", 3, t) for t in range(NT)]
            for qc in (2, 3):
                fl = fillers[qc]
                per_slot = -(-len(fl) // 16)
                fi = 0
                for c in range(4):
                    for hp in range(NHP):
                        if qc == 3:
                            for key in ([("q", 3, hp)] if c == 0 else []):
                                emit_piece(key)
                        emit_attn_turn(qc, hp, c)
                        if c == 3:
                            emit_normalize(qc, hp)
                        if qc == 2 and extras:
                            emit_piece(extras.pop(0))
                        for _ in range(per_slot):
                            if fi < len(fl):
                                wqc, st2, ncol = fl[fi]
                                lowp(piece_wo(wqc, st2, ncol, 512))()
                                fi += 1
                assert fi == len(fl)
            for st2 in range(4):
                for ncol in range(4):
                    lowp(piece_wo(3, st2, ncol, 256))()

    nc.compile()
    return nc


def kernel(query, key, value, w_q, w_k, w_v, w_o):
    import ml_dtypes
    from concourse.bass_utils import run_bass_kernel_spmd

    if "nc" not in _cached:
        _cached["nc"] = _build()
    nc = _cached["nc"]

    bf = ml_dtypes.bfloat16
    query = np.asarray(query, dtype=np.float32)
    key = np.asarray(key, dtype=np.float32)
    value = np.asarray(value, dtype=np.float32)
    w_q = np.asarray(w_q, dtype=np.float32)
    w_k = np.asarray(w_k, dtype=np.float32)
    w_v = np.asarray(w_v, dtype=np.float32)
    w_o = np.asarray(w_o, dtype=np.float32)

    def c(a):
        return np.ascontiguousarray(a).astype(bf)

    in_maps = []
    for core in range(N_CORES):
        b, g = core // G, core % G
        rows = slice(DV * g, DV * (g + 1))
        in_maps.append({
            "xqT": c(query[b].T),
            "xkT": c(key[b].T),
            "xvT": c(value[b].T),
            "wqT": c(w_q[rows, :].T),
            "wkT": c(w_k[rows, :].T),
            "wvT": c(w_v[rows, :].T),
            "woT": c(w_o[:, rows].T),
        })

    res = run_bass_kernel_spmd(nc, in_maps, list(range(N_CORES)))
    full = np.empty((B, S, D), np.float32)
    for b in range(B):
        full[b] = res.results[G * b]["out"] + res.results[G * b + 1]["out"]
    return full


# revision 13
# speedup vs baseline: 1.0378x; 1.0378x over previous
"""Multi-head attention (B=4, S=2048, D=1024, H=16) on 8 TRN2 NeuronCores.

Sharding: data-parallel over batch (4) x tensor-parallel over head halves (2).
Core c handles batch b = c//2 and heads [8g, 8g+8) where g = c%2.
Each core computes a partial [S, D] output-projection contribution; the host
sums the two head-group partials per batch.

Layout (all matmul operands bf16, fp32 PSUM accumulation):
  - activations arrive host-transposed (feature dim on partitions),
  - scores are built transposed [k, q]; one PSUM tile [128, 1024] holds the
    scores of BOTH heads of a pair ([k, head0 q | head1 q]) so the two
    DK=64 score matmuls (PE row halves 0-63 / 64-127, tile_position (0,0)
    and (64,0)) become ready together, schedule back-to-back, and run
    CONCURRENTLY on the PE array (row tiling),
  - exp() covers both heads in one [128, 1024] activation instruction,
  - P @ [V | 1] yields the softmax denominator as row 64 of U^T for free,
  - normalized U^T tiles are exactly the stationary layout w_o needs.

Scheduling: the attention stream (score MMs -> exp -> PV MMs) is emitted at
normal priority; every projection/output-projection "piece" is emitted at
LOW priority (tc.high_priority with a negative offset) so the Tile
scheduler treats pieces as pure PE gap-fillers — the scalar engine's exp
stream (~266us busy) never waits behind piece work.

Phase A processes q-chunks {0,1} interleaved c-major so the K/V projection
of chunks 1-3 fits under 2x of exp cover; phase B runs q-chunks 2,3
qc-major with the w_o pieces of earlier chunks as fillers. The final
chunk's w_o runs at quarter width (N=256) to shorten the serial tail.

PSUM budget (8 banks): 2 x sc [128,1024] (4 banks) + 2 x Up [65,512]
(2 banks) + 2 x piece accumulators [128,512] (2 banks).
"""

import numpy as np

B, S, D, H = 4, 2048, 1024, 16
DK = D // H          # 64
G = 2                # head groups (tensor-parallel degree per batch)
HL = H // G          # 8 local heads per core
DV = HL * DK         # 512 local value dim
N_CORES = 8

LOWP = -1_000_000    # priority offset for gap-filler pieces

_cached = {}


def _build():
    import concourse.bass as bass
    import concourse.tile as tile
    from concourse import bacc, mybir

    f32 = mybir.dt.float32
    bf16 = mybir.dt.bfloat16
    EXP = mybir.ActivationFunctionType.Exp

    nc = bacc.Bacc("TRN2", target_bir_lowering=False, debug=False,
                   num_devices=N_CORES)

    scr = nc.dram_tensor("scr", [128, 512], f32, kind="Internal").ap()
    xqT = nc.dram_tensor("xqT", [D, S], bf16, kind="ExternalInput").ap()
    xkT = nc.dram_tensor("xkT", [D, S], bf16, kind="ExternalInput").ap()
    xvT = nc.dram_tensor("xvT", [D, S], bf16, kind="ExternalInput").ap()
    wqT = nc.dram_tensor("wqT", [D, DV], bf16, kind="ExternalInput").ap()
    wkT = nc.dram_tensor("wkT", [D, DV], bf16, kind="ExternalInput").ap()
    wvT = nc.dram_tensor("wvT", [D, DV], bf16, kind="ExternalInput").ap()
    woT = nc.dram_tensor("woT", [DV, D], bf16, kind="ExternalInput").ap()
    out = nc.dram_tensor("out", [S, D], f32, kind="ExternalOutput").ap()

    ND = D // 128     # 8 d-tiles
    NS = S // 128     # 16 k-tiles
    NQC = S // 512    # 4 q-chunks
    NT = DV // 128    # 4 dk/dv-tiles
    NHP = HL // 2     # 4 head pairs

    with tile.TileContext(nc) as tc:
        with (
            tc.tile_pool(name="persist", bufs=1) as persist,
            tc.tile_pool(name="stage", bufs=32) as stage,
            tc.tile_pool(name="wpool", bufs=8) as wpool,
            tc.tile_pool(name="spool", bufs=2, space=bass.MemorySpace.PSUM) as spool,
            tc.tile_pool(name="upool", bufs=2, space=bass.MemorySpace.PSUM) as upool,
            tc.tile_pool(name="gpool", bufs=2, space=bass.MemorySpace.PSUM) as gpool,
            tc.tile_pool(name="ppool", bufs=4) as ppool,
            tc.tile_pool(name="rpool", bufs=3) as rpool,
            tc.tile_pool(name="obuf", bufs=3) as obuf,
        ):
            QT = {}    # [t][qc] -> [128, 512] tiles of Q^T (dk rows, q cols)
            KT = {}    # [t][c]  -> [128, 512]
            Vaug = {}  # [kt] -> [128, 8, 65]: per-head V columns + ones col
            outT = {}  # [qc][t] -> [128, 512] normalized attention out^T
            wks, wvs, wqs, wos = [], [], [], []
            st_ = {}   # per (qc, hp) attention state
            xq_stage = {}

            def uacc():
                return upool.tile([65, 512], f32, tag="u", name="uacc")

            def gacc():
                return gpool.tile([128, 512], f32, tag="g", name="gacc")

            def emit_w_loads(lst, name, src):
                for d in range(ND):
                    wt = wpool.tile([128, DV], bf16, tag="w", name=name,
                                    bufs=24)
                    nc.sync.dma_start(wt[:], src[128 * d:128 * (d + 1), :])
                    lst.append(wt)

            def emit_wo_loads():
                for t in range(NT):
                    wo = wpool.tile([128, D], bf16, tag=f"wo{t}", name="wo",
                                    bufs=1)
                    nc.sync.dma_start(wo[:], woT[128 * t:128 * (t + 1), :])
                    wos.append(wo)

            def emit_x_dmas(src, c, name):
                xs = []
                for d in range(ND):
                    xt = stage.tile([128, 512], bf16, tag="act", name=name)
                    nc.sync.dma_start(
                        xt[:], src[128 * d:128 * (d + 1), 512 * c:512 * (c + 1)])
                    xs.append(xt)
                return xs

            def piece_kproj(xks, c, t):
                def go():
                    acc = gacc()
                    for d in range(ND):
                        nc.tensor.matmul(
                            acc[:], wks[d][:, 128 * t:128 * (t + 1)], xks[d][:],
                            start=(d == 0), stop=(d == ND - 1))
                    dt_ = persist.tile([128, 512], bf16, tag=f"kT{t}_{c}",
                                       name="kT")
                    nc.vector.tensor_copy(dt_[:], acc[:])
                    KT.setdefault(t, {})[c] = dt_
                return go

            def piece_vproj(xvs, c, ktl):
                def go():
                    kt = 4 * c + ktl
                    acc = gacc()
                    for d in range(ND):
                        nc.tensor.matmul(
                            acc[:], xvs[d][:, 128 * ktl:128 * (ktl + 1)],
                            wvs[d][:],
                            start=(d == 0), stop=(d == ND - 1))
                    va = persist.tile([128, HL, DK + 1], bf16, tag=f"vaug{kt}",
                                      name="vaug")
                    nc.vector.tensor_copy(
                        va[:, :, 0:DK],
                        acc[:].rearrange("p (h k) -> p h k", h=HL))
                    nc.vector.tensor_copy(
                        va[:, :, DK], nc.const_aps.tensor(1.0, (128, HL), bf16))
                    Vaug[kt] = va
                return go

            def emit_xq_dmas(qc):
                xs = []
                for d in range(ND):
                    xt = stage.tile([128, 512], bf16, tag="act", name="xq")
                    nc.sync.dma_start(
                        xt[:], xqT[128 * d:128 * (d + 1), 512 * qc:512 * (qc + 1)])
                    xs.append(xt)
                xq_stage[qc] = xs

            def piece_qproj(qc, t):
                def go():
                    xs = xq_stage[qc]
                    acc = gacc()
                    for d in range(ND):
                        nc.tensor.matmul(
                            acc[:], wqs[d][:, 128 * t:128 * (t + 1)], xs[d][:],
                            start=(d == 0), stop=(d == ND - 1))
                    dt_ = persist.tile([128, 512], bf16, tag=f"qT{t}_{qc}",
                                       name="qT")
                    nc.vector.tensor_copy(dt_[:], acc[:])
                    QT.setdefault(t, {})[qc] = dt_
                return go

            wo_stash = {}

            def piece_wo(qc, st, ncols, half=None):
                # final[s, n] = sum_dv outT[dv, s] * woT[dv, n].  half=0
                # contracts head-pair tiles {0,1} into an SBUF stash (can
                # run before the last normalize); half=1 adds tiles {2,3}.
                def go():
                    acc = gacc()
                    ts = {None: range(NT), 0: (0, 1), 1: (2, 3)}[half]
                    for j, t in enumerate(ts):
                        nc.tensor.matmul(
                            acc[:],
                            outT[qc][t][:, 128 * st:128 * (st + 1)],
                            wos[t][:, 512 * ncols:512 * (ncols + 1)],
                            start=(j == 0), stop=(j == len(ts) - 1))
                    if half == 0:
                        sb = obuf.tile([128, 512], f32, tag="stash",
                                       name="stash", bufs=8)
                        nc.vector.tensor_copy(sb[:], acc[:])
                        wo_stash[(qc, st, ncols)] = sb
                        return
                    ob = obuf.tile([128, 512], f32, tag="ob", name="ob")
                    if half == 1:
                        nc.vector.tensor_add(
                            ob[:], wo_stash[(qc, st, ncols)][:], acc[:])
                    else:
                        nc.vector.tensor_copy(ob[:], acc[:])
                    nc.sync.dma_start(
                        out[512 * qc + 128 * st:512 * qc + 128 * (st + 1),
                            512 * ncols:512 * (ncols + 1)],
                        ob[:])
                return go

            def lowp(fn):
                def go():
                    with tc.high_priority(offset=LOWP):
                        fn()
                return go

            def emit_attn_turn(qc, hp, c):
                # head pair (2hp, 2hp+1) = partition halves of tile hp. One
                # sc tile [128 kpos, head0 q | head1 q] per k-tile: the two
                # DK=64 score matmuls share the tile (ready together ->
                # adjacent in the PE queue -> concurrent row tiles).
                # PV matmuls are emitted AFTER the next k-tile's score pair
                # (software pipelining): the score pair then outranks a
                # just-became-ready PV in the scheduler, so pairs stay
                # adjacent and PV never parks the PE queue on its exp.
                t = hp
                s = st_.setdefault((qc, hp), {})
                if c == 0:
                    s["Usb"] = [rpool.tile([65, 512], f32, tag=f"usb{hp}_{i}",
                                           name="usb", bufs=2)
                                for i in range(2)]
                Up = [None, None]
                pend = []

                def flush_pv():
                    P, kt = pend.pop(0)
                    ktl = kt - 4 * c
                    for i in range(2):
                        nc.tensor.matmul(
                            Up[i][:],
                            Vaug[kt][:, 2 * hp + i, :],
                            P[:, 512 * i:512 * (i + 1)],
                            start=(ktl == 0), stop=(ktl == 3))

                for ktl in range(4):
                    kt = 4 * c + ktl
                    sc = spool.tile([128, 1024], f32, tag="sc", name="sc")
                    for i in range(2):
                        po = 64 * i
                        nc.tensor.matmul(
                            sc[:, 512 * i:512 * (i + 1)],
                            KT[t][c][po:po + 64,
                                     128 * ktl:128 * (ktl + 1)],
                            QT[t][qc][po:po + 64, :],
                            start=True, stop=True)
                    P = ppool.tile([128, 1024], bf16, tag="p", name="p")
                    nc.scalar.activation(P[:], sc[:], EXP, scale=0.125)
                    if ktl == 0:
                        Up[0] = uacc()
                        Up[1] = uacc()
                    pend.append((P, kt))
                    if ktl >= 1:
                        flush_pv()
                flush_pv()
                for i in range(2):
                    if c == 0:
                        nc.vector.tensor_copy(s["Usb"][i][:], Up[i][:])
                    else:
                        nc.vector.tensor_add(s["Usb"][i][:],
                                             s["Usb"][i][:], Up[i][:])

            def emit_normalize(qc, hp):
                # rows 0..63 of U divided by row 64 (the ones-column sum),
                # written into out^T. Engine ops keep operands on one
                # partition range; cross-partition moves via SBUF-SBUF DMA.
                t = hp
                Usb = st_[(qc, hp)]["Usb"]
                ot = persist.tile([128, 512], bf16, tag=f"oT{t}_{qc % 2}",
                                  name="oT")
                outT.setdefault(qc, {})[t] = ot
                for i in range(2):
                    rrow = rpool.tile([1, 512], f32, tag="rrow", name="rrow")
                    nc.sync.dma_start(rrow[:], Usb[i][64:65, :])
                    rrec = rpool.tile([1, 512], f32, tag="rrec", name="rrec")
                    nc.vector.reciprocal_approx_fast(rrec[:], rrow[:])
                    rb = rpool.tile([64, 512], f32, tag="rb", name="rb")
                    nc.gpsimd.partition_broadcast(rb[:], rrec[:])
                    if i == 0:
                        nc.vector.tensor_mul(ot[0:64, :], Usb[i][0:64, :],
                                             rb[:])
                    else:
                        stg = rpool.tile([64, 512], bf16, tag="stg",
                                         name="stg")
                        nc.vector.tensor_mul(stg[:], Usb[i][0:64, :], rb[:])
                        nc.sync.dma_start(ot[64:128, :], stg[:])

            # ---- warm-up: load the exp table + flip the PE HAM to full
            # clock during the initial DMA wait, using a zeroed SBUF tile.
            # The exp reads the warm matmul's PSUM and the result lands in
            # `out` (overwritten later) so nothing here is dead code.
            wtile = stage.tile([128, 512], bf16, tag="warm", name="warm",
                               bufs=1)
            nc.vector.memset(wtile[:], 0.0)
            wacc = gacc()
            for r in range(16):
                nc.tensor.matmul(wacc[:], wtile[:, 0:128], wtile[:],
                                 start=(r == 0), stop=(r == 15))
            wexp = stage.tile([128, 512], f32, tag="warm2", name="warm2",
                              bufs=1)
            nc.scalar.activation(wexp[:], wacc[:], EXP, scale=0.125)

            # ---- DMA order: V path first (first PE work), then K, Q.
            emit_w_loads(wvs, "wv", wvT)
            xvs0 = emit_x_dmas(xvT, 0, "xv")
            emit_w_loads(wks, "wk", wkT)
            xks0 = emit_x_dmas(xkT, 0, "xk")
            emit_w_loads(wqs, "wq", wqT)
            emit_xq_dmas(0)
            emit_xq_dmas(1)

            xstage = {0: (xks0, xvs0)}

            # Pieces keyed for just-in-time emission before the turn that
            # first reads their tile (a Python-level ordering requirement;
            # execution order is still dependency + priority driven).
            emitted = set()

            def emit_piece(key):
                if key in emitted:
                    return
                emitted.add(key)
                kind = key[0]
                if kind == "v":
                    _, c, ktl = key
                    lowp(piece_vproj(xstage[c][1], c, ktl))()
                elif kind == "k":
                    _, c, t = key
                    lowp(piece_kproj(xstage[c][0], c, t))()
                elif kind == "q":
                    _, qc, t = key
                    lowp(piece_qproj(qc, t))()

            def turn_needs(qc, hp, c):
                # kproj first: at a chunk boundary the scores of the next
                # chunk only need K^T, so it should outrank the V pieces.
                return ([("k", c, hp), ("q", qc, hp)]
                        + [("v", c, ktl) for ktl in range(4)])

            # ---- prologue: minimal deps of the first attention turn at
            # normal priority; the other three Vaug tiles follow low-prio.
            piece_vproj(xvs0, 0, 0)()
            piece_kproj(xks0, 0, 0)()
            piece_qproj(0, 0)()
            emitted |= {("v", 0, 0), ("k", 0, 0), ("q", 0, 0)}

            # ---- phase A: q-chunks {0, 1} interleaved, c-major, so the
            # K/V projection of chunks 1-3 sits under 2x of exp cover.
            extras = [("q", 2, t) for t in range(NT)]
            for c in range(4):
                if c < 3:
                    xstage[c + 1] = (emit_x_dmas(xkT, c + 1, "xk"),
                                     emit_x_dmas(xvT, c + 1, "xv"))
                if c == 1:
                    emit_wo_loads()
                    emit_xq_dmas(2)
                    emit_xq_dmas(3)
                for hp in range(NHP):
                    for qc in (0, 1):
                        for key in turn_needs(qc, hp, c):
                            emit_piece(key)
                        emit_attn_turn(qc, hp, c)
                        if c == 3:
                            emit_normalize(qc, hp)
                        if c >= 2 and extras:
                            emit_piece(extras.pop(0))

            # ---- phase B: q-chunks 2 then 3, hp-major (a head pair's
            # normalize lands as soon as its four c-groups finish, so the
            # final chunk's w_o half-pieces over head tiles {0,1} overlap
            # the remaining attention); w_o of earlier chunks as fillers.
            extras = [("q", 3, t) for t in range(NT)]
            for qc in (2, 3):
                for hp in range(NHP):
                    emit_piece(("q", qc, hp))
                    for c in range(4):
                        emit_attn_turn(qc, hp, c)
                        if c == 3:
                            emit_normalize(qc, hp)
                        if qc == 2 and extras:
                            emit_piece(extras.pop(0))
                    if qc == 3 and hp == 1:
                        # outT[3][0..1] exist: half-0 w_o of the final
                        # chunk overlaps the remaining attention.
                        for st2 in range(4):
                            for ncol in range(2):
                                lowp(piece_wo(3, st2, ncol, 0))()
                if qc == 2:
                    for wqc in (0, 1, 2):
                        for st2 in range(4):
                            for ncol in range(2):
                                lowp(piece_wo(wqc, st2, ncol, None))()
            for st2 in range(4):
                for ncol in range(2):
                    lowp(piece_wo(3, st2, ncol, 1))()

            # warm-exp sink: a scratch-DRAM store, last in every queue.
            with tc.high_priority(offset=LOWP * 2):
                nc.sync.dma_start(scr[:, :], wexp[:])

    nc.compile()
    return nc


def kernel(query, key, value, w_q, w_k, w_v, w_o):
    import ml_dtypes
    from concourse.bass_utils import run_bass_kernel_spmd

    if "nc" not in _cached:
        _cached["nc"] = _build()
    nc = _cached["nc"]

    bf = ml_dtypes.bfloat16
    query = np.asarray(query, dtype=np.float32)
    key = np.asarray(key, dtype=np.float32)
    value = np.asarray(value, dtype=np.float32)
    w_q = np.asarray(w_q, dtype=np.float32)
    w_k = np.asarray(w_k, dtype=np.float32)
    w_v = np.asarray(w_v, dtype=np.float32)
    w_o = np.asarray(w_o, dtype=np.float32)

    def c(a):
        return np.ascontiguousarray(a).astype(bf)

    in_maps = []
    for core in range(N_CORES):
        b, g = core // G, core % G
        rows = slice(DV * g, DV * (g + 1))
        in_maps.append({
            "xqT": c(query[b].T),
            "xkT": c(key[b].T),
            "xvT": c(value[b].T),
            "wqT": c(w_q[rows, :].T),
            "wkT": c(w_k[rows, :].T),
            "wvT": c(w_v[rows, :].T),
            "woT": c(w_o[:, rows].T),
        })

    res = run_bass_kernel_spmd(nc, in_maps, list(range(N_CORES)))
    full = np.empty((B, S, D), np.float32)
    for b in range(B):
        full[b] = res.results[G * b]["out"] + res.results[G * b + 1]["out"]
    return full
